# revision 30
# baseline (speedup 1.0000x reference)
"""AU-guided attention fusion kernel for 8 Trainium2 NeuronCores.

v3 strategy (data parallel over batch, weights replicated):

Built on v2's algebraic restructuring (k/v collapse to affine-in-au form,
LN rsqrt via exp/ln, folded projections, host-computed r/tr). Changes:

  - 2-step software pipeline: phase_a(t+2)'s matmuls are issued in chunks
    BETWEEN phase_b(t)'s j-blocks, so the PE's in-order queue never waits
    on phase_a's serial PE->ACT->DVE chains (v2 lost ~8.5us/tile there).
    Bootstrap interleaves phase_a(0) and phase_a(1) pairwise.
  - Wg1 hybrid precision: K-blocks 0,1 run as one fp8e4 DoubleRow matmul
    (weights+x fp8), blocks 2-5 stay bf16. Saves 1 PE pass per j-block.
    Measured rel err 1.18e-2 vs the 2e-2 gate (4-block fp8 sims at
    1.86e-2 and full fp8 at 1.97e-2 -- too close to the gate).
  - Deferred drains: each j-block's pd matmuls + den/gj/stt run one
    j-block later than its pg matmuls, shortening PSUM bank residency.
  - Engine placement by measured per-op cost (GpSimd tensor_scalar is
    14.5ns/col -- never use; GpSimd tensor_tensor 2.26ns/col; ACT/DVE
    ~1.3ns/col): den on ACT, recip/stt on DVE, skip-adds on GpSimd,
    input DMA triggers on SP+ACT queues, output DMA on SP.
  - DMA issue order: wq8/cvec/tile0 first so the first matmul starts
    ~3us earlier; 4-tile input prefetch depth.
  - PSUM: pfa 2 + pfb 2 + pjl 4 banks (two j-blocks in flight).

Schedule notes from measurement (HW ~164us, down from v2's 181us): the
engines sit at PE 15.0 / ACT 11.8 / DVE 11.3 / Pool 10.3 us busy per
~20.5us tile-step. Timing is bimodal: ~164us typical with a rare ~194us
slow mode (environment noise) -- evaluate schedule changes with 3+ runs.
Bootstrap prestarts tile-0's gate x-matmuls (wg18/wg1b shipped early on
the ACT DMA queue so they arrive in time). Many plausible-looking
reorderings (earlier a_var, a_start hoisting, single front PSUM tag
with pjl=5, den on DVE or split ACT/DVE, batched bf16 adds on DVE,
est-fp8 DR for pnm) all measured WORSE.
"""

import numpy as np

import concourse.bass as bass
import concourse.tile as tile
from concourse import bacc, mybir
from concourse.bass import ts
from concourse.bass_utils import run_bass_kernel_spmd

B, D, A, H, NH = 32768, 768, 17, 128, 4
DH = H // NH
NCORES = 8
BC = B // NCORES          # 4096 rows per core
NB = 512                  # batch columns per tile
NT = BC // NB             # 8 tiles per core
KD = D // H               # 6 feature blocks of 128
A4 = NH * A               # 68
EPS = 1e-5
QSCL = 256.0              # host scale on Wq_eff for fp8
GSCL = 32.0               # host scale on Gv/Gu/Gc/Gd
WGS = 64.0                # host scale on Wg1/W2eff (gate logit path)
NF8 = 2                   # Wg1 K-blocks in fp8 (one DoubleRow pair)

F32 = mybir.dt.float32
BF16 = mybir.dt.bfloat16
F8 = mybir.dt.float8e4
AF = mybir.ActivationFunctionType
OP = mybir.AluOpType
DR = mybir.MatmulPerfMode.DoubleRow

ACT_SET = "natural_log_exp_and_others"


def _pin_act_tables():
    """Pin every used activation into the one table set with exp AND ln."""
    used = {AF.Exp, AF.Ln, AF.Square, AF.Identity, AF.Copy}
    prev = bacc.get_activation_tables

    def patched(arch):
        tabs = dict(prev(arch))
        return {
            name: (set(fns) if name == ACT_SET else set(fns) - used)
            for name, fns in tabs.items()
        }

    bacc.get_activation_tables = patched
    return prev


def build_bass():
    nc = bacc.Bacc("TRN2", target_bir_lowering=False, debug=False,
                   num_devices=NCORES)

    # tile-major layouts: one contiguous segment per partition per tile
    xTb = nc.dram_tensor("xTb", [H, NT, KD, NB], BF16, kind="ExternalInput")
    xT8 = nc.dram_tensor("xT8", [H, NT, KD, NB], F8, kind="ExternalInput")
    rta = nc.dram_tensor("rta", [A4, NT, 3, NB], BF16, kind="ExternalInput")
    wqe8 = nc.dram_tensor("wqe8", [H, KD, H], F8, kind="ExternalInput")
    mamb = nc.dram_tensor("mamb", [H, 2 * A4], BF16, kind="ExternalInput")
    gvu = nc.dram_tensor("gvu", [A4, 2 * H], BF16, kind="ExternalInput")
    gcd = nc.dram_tensor("gcd", [A4, 2 * H], BF16, kind="ExternalInput")
    waoT = nc.dram_tensor("waoT", [H, H], BF16, kind="ExternalInput")
    woutT = nc.dram_tensor("woutT", [H, D], BF16, kind="ExternalInput")
    w2eT = nc.dram_tensor("w2eT", [H, D], BF16, kind="ExternalInput")
    wg18 = nc.dram_tensor("wg18", [H, KD, NF8, H], F8, kind="ExternalInput")
    wg1b = nc.dram_tensor("wg1b", [H, KD, KD - NF8, H], BF16,
                          kind="ExternalInput")
    negI = nc.dram_tensor("negI", [H, H], BF16, kind="ExternalInput")
    # cvec cols: 0 bq_eff | 1 g_aln | 2 bhat*g_aln | 3:9 bfin_j | 9:15 -bg_j
    cvec = nc.dram_tensor("cvec", [H, 16], F32, kind="ExternalInput")
    wvard = nc.dram_tensor("wvard", [H, 1], BF16, kind="ExternalInput")
    onesd = nc.dram_tensor("onesd", [1, H], BF16, kind="ExternalInput")

    outT = nc.dram_tensor("outT", [H, NT, KD, NB], BF16,
                          kind="ExternalOutput")

    with tile.TileContext(nc) as tc:
        with (
            tc.tile_pool(name="consts", bufs=1) as cons,
            tc.tile_pool(name="pvis", bufs=5) as pvis,
            tc.tile_pool(name="p68", bufs=5) as p68,
            tc.tile_pool(name="pfr", bufs=3) as pfr,   # front transients
            tc.tile_pool(name="pqs", bufs=2) as pqs,
            tc.tile_pool(name="pctx", bufs=2) as pctx,
            tc.tile_pool(name="pchg", bufs=2) as pchg,
            tc.tile_pool(name="plng", bufs=2) as plng,
            tc.tile_pool(name="pej", bufs=3) as pej,
            tc.tile_pool(name="pgt", bufs=3) as pgt,
            tc.tile_pool(name="pmj", bufs=2) as pmj,
            tc.tile_pool(name="pout", bufs=2) as pout,
            tc.tile_pool(name="psum", bufs=1, space="PSUM") as ps,
        ):
            # ---- constants: ordered so the first matmuls start ASAP ----
            wq8_sb = cons.tile([H, KD, H], F8)
            nc.sync.dma_start(out=wq8_sb, in_=wqe8[:, :, :])
            cvec_sb = cons.tile([H, 16], F32)
            nc.scalar.dma_start(out=cvec_sb, in_=cvec[:, :])
            wg18_sb = cons.tile([H, KD, NF8, H], F8)
            nc.scalar.dma_start(out=wg18_sb, in_=wg18[:, :, :, :])
            wg1b_sb = cons.tile([H, KD, KD - NF8, H], BF16)
            nc.scalar.dma_start(out=wg1b_sb[:, 0:3, :, :],
                                in_=wg1b[:, 0:3, :, :])
            nc.scalar.dma_start(out=wg1b_sb[:, 3:6, :, :],
                                in_=wg1b[:, 3:6, :, :])

            def load_tile(t):
                v8 = pvis.tile([H, KD, NB], F8, tag="vis8")
                nc.sync.dma_start(out=v8, in_=xT8[:, t, :, :])
                rt = p68.tile([A4, 3, NB], BF16, tag="rtr")
                nc.sync.dma_start(out=rt, in_=rta[:, t, :, :])
                vb = pvis.tile([H, KD, NB], BF16, tag="visb")
                nc.scalar.dma_start(out=vb, in_=xTb[:, t, :, :])
                return v8, rt, vb

            pre = {0: load_tile(0)}
            mamb_sb = cons.tile([H, 2 * A4], BF16)
            nc.sync.dma_start(out=mamb_sb, in_=mamb[:, :])
            pre[1] = load_tile(1)
            gvu_sb = cons.tile([A4, 2 * H], BF16)
            nc.sync.dma_start(out=gvu_sb, in_=gvu[:, :])
            gcd_sb = cons.tile([A4, 2 * H], BF16)
            nc.sync.dma_start(out=gcd_sb, in_=gcd[:, :])
            wao_sb = cons.tile([H, H], BF16)
            nc.sync.dma_start(out=wao_sb, in_=waoT[:, :])
            wout_sb = cons.tile([H, D], BF16)
            nc.scalar.dma_start(out=wout_sb, in_=woutT[:, :])
            w2e_sb = cons.tile([H, D], BF16)
            nc.scalar.dma_start(out=w2e_sb, in_=w2eT[:, :])
            negI_sb = cons.tile([H, H], BF16)
            nc.scalar.dma_start(out=negI_sb, in_=negI[:, :])
            wvar_sb = cons.tile([H, 1], BF16)
            nc.sync.dma_start(out=wvar_sb, in_=wvard[:, :])
            ones_sb = cons.tile([1, H], BF16)
            nc.sync.dma_start(out=ones_sb, in_=onesd[:, :])
            eps_sb = cons.tile([1, 1], F32)
            nc.vector.memset(eps_sb, EPS)
            ones512 = cons.tile([H, NB], BF16)
            nc.gpsimd.memset(ones512, 1.0)
            pre[2] = load_tile(2)
            pre[3] = load_tile(3)

            # phase_a state per tile index
            S = {}

            def a_start(u):
                vis8, rtr, visb = pre.pop(u)
                psq = ps.tile([H, NB], F32, tag="pfa", bufs=2)
                for i in range(KD // 2):
                    nc.tensor.matmul(psq, wq8_sb[:, 2 * i:2 * i + 2, :],
                                     vis8[:, 2 * i:2 * i + 2, :],
                                     start=(i == 0), stop=(i == 2),
                                     perf_mode=DR)
                qs = pqs.tile([H, NB], BF16, tag="qs")
                nc.scalar.activation(qs, psq, AF.Identity,
                                     scale=1.0 / QSCL,
                                     bias=cvec_sb[:, 0:1])
                S[u] = dict(vis8=vis8, rtr=rtr, visb=visb, qs=qs)

            def a_scores(u):
                s = S[u]
                psa = ps.tile([A4, NB], F32, tag="pfa", bufs=2)
                nc.tensor.matmul(psa, mamb_sb[:, 0:A4], s["qs"])
                psb = ps.tile([A4, NB], F32, tag="pfa", bufs=2)
                nc.tensor.matmul(psb, mamb_sb[:, A4:2 * A4], s["qs"])
                rtr = s["rtr"]
                x1 = pfr.tile([A4, NB], F32, tag="x1")
                nc.vector.tensor_mul(x1, psa, rtr[:, 2, :])
                x2 = pfr.tile([A4, NB], F32, tag="x2")
                nc.vector.tensor_add(x2, x1, psb)
                sc = pfr.tile([A4, NB], F32, tag="sc")
                nc.gpsimd.tensor_mul(sc, x2, rtr[:, 0, :])
                s["sc"] = sc

            def a_soft(u):
                s = S[u]
                Ee = pfr.tile([A4, NB], BF16, tag="Ee")
                nc.scalar.activation(Ee, s["sc"], AF.Exp)
                est = pfr.tile([A4, 2, NB], BF16, tag="est")
                nc.gpsimd.tensor_mul(est[:, 0, :], Ee, s["rtr"][:, 0, :])
                nc.gpsimd.tensor_mul(est[:, 1, :], Ee, s["rtr"][:, 1, :])
                s["Ee"] = Ee
                s["est"] = est

            def a_comb(u):
                s = S[u]
                pnm = ps.tile([H, NB], F32, tag="pfb", bufs=2)
                nc.tensor.matmul(pnm, gvu_sb[:, 0:H], s["est"][:, 0, :],
                                 start=True, stop=False)
                nc.tensor.matmul(pnm, gvu_sb[:, H:2 * H], s["est"][:, 1, :],
                                 start=False, stop=False)
                nc.tensor.matmul(pnm, gcd_sb[:, 0:H], s["Ee"],
                                 start=False, stop=True)
                pdn = ps.tile([H, NB], F32, tag="pfb", bufs=2)
                nc.tensor.matmul(pdn, gcd_sb[:, H:2 * H], s["Ee"])
                rd = pfr.tile([H, NB], F32, tag="rd")
                nc.vector.reciprocal_approx_fast(out=rd, in_=pdn)
                ctx = pctx.tile([H, NB], BF16, tag="ctx")
                nc.vector.tensor_mul(ctx, pnm, rd)
                s["ctx"] = ctx

            def a_ao(u):
                s = S[u]
                pao = ps.tile([H, NB], F32, tag="pfb", bufs=2)
                nc.tensor.matmul(pao, wao_sb, s["ctx"])
                chg = pchg.tile([H, NB], F32, tag="chg")
                nc.scalar.activation(chg, pao, AF.Identity,
                                     scale=cvec_sb[:, 1:2],
                                     bias=cvec_sb[:, 2:3])
                c2 = pfr.tile([H, NB], BF16, tag="c2")
                nc.scalar.activation(c2, chg, AF.Square)
                s["chg"] = chg
                s["c2"] = c2

            def a_var(u):
                s = S[u]
                pvar = ps.tile([1, NB], F32, tag="pfb", bufs=2)
                nc.tensor.matmul(pvar, wvar_sb[:, 0:1], s["c2"])
                lv = pfr.tile([1, NB], F32, tag="lv")
                nc.scalar.activation(lv, pvar, AF.Ln, bias=eps_sb[:, 0:1])
                rv = pfr.tile([1, NB], BF16, tag="rv")
                nc.scalar.activation(rv, lv, AF.Exp, scale=-0.5)
                s["rv"] = rv

            def a_fin(u):
                s = S[u]
                prs = ps.tile([H, NB], F32, tag="pfb", bufs=2)
                nc.tensor.matmul(prs, ones_sb[0:1, :], s["rv"])
                lng = plng.tile([H, NB], BF16, tag="lng")
                nc.vector.tensor_mul(lng, s["chg"], prs)
                s["lng"] = lng

            GJ = {}
            PG = {}

            def b_pg_x(t, j):
                s = S[t]
                visb, vis8 = s["visb"], s["vis8"]
                pg = ps.tile([H, NB], F32, tag="pjl", bufs=4)
                nc.tensor.matmul(pg, wg18_sb[:, j, :, :],
                                 vis8[:, 0:NF8, :],
                                 start=True, stop=False, perf_mode=DR)
                for i in range(NF8, KD):
                    nc.tensor.matmul(pg, wg1b_sb[:, j, i - NF8, :],
                                     visb[:, i, :], start=False, stop=False)
                PG[(t, j)] = pg

            def b_pg(t, j):
                if (t, j) not in PG:
                    b_pg_x(t, j)
                pg = PG.pop((t, j))
                s = S[t]
                jb = ts(j, H)
                nc.tensor.matmul(pg, w2e_sb[:, jb], s["lng"],
                                 start=False, stop=True)
                ej = pej.tile([H, NB], BF16, tag="ej")
                nc.scalar.activation(ej, pg, AF.Exp, scale=-1.0 / WGS,
                                     bias=cvec_sb[:, 9 + j:10 + j])
                GJ[(t, j)] = ej

            def b_pd(t, j, mj6, ot6):
                s = S[t]
                visb, lng = s["visb"], s["lng"]
                jb = ts(j, H)
                ej = GJ.pop((t, j))
                den = pgt.tile([H, NB], F32, tag="den")
                if j >= 4:
                    nc.gpsimd.tensor_add(den, ej, ones512)
                else:
                    nc.scalar.activation(den, ej, AF.Identity, bias=1.0)
                gj = pgt.tile([H, NB], F32, tag="gj")
                nc.vector.reciprocal_approx_fast(out=gj, in_=den)
                pd = ps.tile([H, NB], F32, tag="pjl", bufs=4)
                nc.tensor.matmul(pd, wout_sb[:, jb], lng,
                                 start=True, stop=False)
                nc.tensor.matmul(pd, negI_sb, visb[:, j, :],
                                 start=False, stop=True)
                nc.vector.scalar_tensor_tensor(
                    mj6[:, j, :], pd, cvec_sb[:, 3 + j:4 + j], gj,
                    op0=OP.add, op1=OP.mult)
                nc.gpsimd.tensor_add(ot6[:, j, :], visb[:, j, :],
                                     mj6[:, j, :])

            # ---- bootstrap: phase_a(0)+(1) interleaved pairwise ----
            a_start(0); a_start(1)
            a_scores(0); a_scores(1)
            a_soft(0); b_pg_x(0, 0); a_soft(1)
            b_pg_x(0, 1)
            a_comb(0); b_pg_x(0, 2); a_comb(1)
            a_ao(0); b_pg_x(0, 3); a_ao(1)
            a_var(0); a_var(1)
            a_fin(0); a_fin(1)

            for t in range(NT):
                if t + 4 < NT:
                    pre[t + 4] = load_tile(t + 4)
                u = t + 2
                doA = u < NT
                mj6 = pmj.tile([H, KD, NB], BF16, tag="mj6")
                ot = pout.tile([H, KD, NB], BF16, tag="ot")
                visb = S[t]["visb"]

                if doA:
                    a_start(u)
                b_pg(t, 0)
                if doA:
                    a_scores(u)
                b_pg(t, 1)
                b_pd(t, 0, mj6, ot)
                if doA:
                    a_soft(u)
                b_pg(t, 2)
                b_pd(t, 1, mj6, ot)
                if doA:
                    a_comb(u)
                b_pg(t, 3)
                b_pd(t, 2, mj6, ot)
                nc.sync.dma_start(out=outT[:, t, 0:3, :], in_=ot[:, 0:3, :])
                if doA:
                    a_ao(u)
                b_pg(t, 4)
                b_pd(t, 3, mj6, ot)
                if doA:
                    a_var(u)
                b_pg(t, 5)
                b_pd(t, 4, mj6, ot)
                if doA:
                    a_fin(u)
                b_pd(t, 5, mj6, ot)
                nc.sync.dma_start(out=outT[:, t, 3:6, :], in_=ot[:, 3:6, :])
                del S[t]["vis8"]

    prev = _pin_act_tables()
    try:
        nc.finalize()
    finally:
        bacc.get_activation_tables = prev
    return nc


def host_constants(inputs):
    f = lambda k: np.asarray(inputs[k], np.float64)
    w = f("w_tok")[:, 0]
    pos0 = f("pos")[0]
    cA = f("b_tok")[None, :] + pos0
    mw = w.mean()
    wp = w - mw
    mc = cA.mean(axis=1, keepdims=True)
    cp = cA - mc
    alpha = (wp ** 2).mean()
    beta = 2.0 * (wp[None, :] * cp).mean(axis=1)
    gamma = (cp ** 2).mean(axis=1)
    h_a = beta / (2 * alpha)
    k_a = gamma + EPS - beta ** 2 / (4 * alpha)

    g_au = f("g_auln")
    b_au = f("b_auln")
    wg = wp * g_au
    cg = cp * g_au[None, :]
    w_in = f("w_in")
    b_in = f("b_in")
    wq_, wk_, wv_ = w_in[:H], w_in[H:2 * H], w_in[2 * H:]
    bq_, bk_, bv_ = b_in[:H], b_in[H:2 * H], b_in[2 * H:]
    u_k = wk_ @ wg
    Vk = cg @ wk_.T
    u_v = wv_ @ wg
    Vv = cg @ wv_.T
    cv = wv_ @ b_au + bv_
    scale = 1.0 / np.sqrt(DH)
    Wq_eff = (wq_ @ f("w_q")) * scale
    bq_eff = (wq_ @ f("b_q") + bq_) * scale

    head = np.arange(H) // DH
    colhead = np.repeat(np.arange(NH), A)
    cola = np.tile(np.arange(A), NH)
    mask = (head[:, None] == colhead[None, :]).astype(np.float64)
    MA = u_k[:, None] * mask
    MB = Vk[cola, :].T * mask
    maskT = mask.T
    Gv = Vv[cola, :] * maskT
    Gu = u_v[None, :] * maskT
    Gc = cv[None, :] * maskT
    Gd = maskT

    w_ao = f("w_ao")
    b_ao = f("b_ao")
    m_ao = w_ao.mean(axis=0)
    mb_ao = b_ao.mean()
    What = w_ao - m_ao[None, :]
    bhat = b_ao - mb_ao
    g_aln = f("g_aln")
    b_aln = f("b_aln")
    assert (np.abs(g_aln) > 1e-6).all(), "zero LN gain not supported"
    wvar = 1.0 / (H * g_aln ** 2)
    w_out = f("w_out")
    b_out = f("b_out")
    bfin = w_out @ b_aln + b_out
    w_gate = f("w_gate")
    b_gate = f("b_gate")
    Wg1 = w_gate[:, :D]
    Wg2 = w_gate[:, D:]
    W2eff = Wg2 @ w_out
    bg_eff = b_gate + Wg2 @ b_out + W2eff @ b_aln

    bf = mybir.dt.np(BF16)
    f8 = mybir.dt.np(F8)
    c = lambda x: np.ascontiguousarray(np.asarray(x, np.float32))
    cb = lambda x: np.ascontiguousarray(np.asarray(x, np.float32).astype(bf))
    c8 = lambda x: np.ascontiguousarray(np.asarray(x, np.float32).astype(f8))
    cvec = np.zeros((H, 16), np.float64)
    cvec[:, 0] = bq_eff
    cvec[:, 1] = g_aln
    cvec[:, 2] = bhat * g_aln
    for j in range(KD):
        cvec[:, 3 + j] = bfin[j * H:(j + 1) * H]
        cvec[:, 9 + j] = -bg_eff[j * H:(j + 1) * H]

    assert np.abs(Wq_eff * QSCL).max() < 200, "fp8 overflow in Wq"
    assert max(np.abs(Gv * GSCL).max(), np.abs(Gu * GSCL).max()) < 200, \
        "fp8 overflow in Gv/Gu"
    assert np.abs(Wg1 * WGS).max() < 200, "fp8 overflow in Wg1"

    gcd = np.concatenate([Gc * GSCL, Gd * GSCL], axis=1)  # [A4, 2H]

    # wqe8 in [k, i, m] layout: input feature f = i*128 + k
    wq_pim = np.ascontiguousarray(
        (Wq_eff.T * QSCL).reshape(KD, H, H).transpose(1, 0, 2))
    # Wg1 [out=768, in=768] -> [j, m, i, k]; lhsT layout [k, j, i, m]
    wg1_4d = (Wg1 * WGS).reshape(KD, H, KD, H)
    wg18 = np.ascontiguousarray(wg1_4d[:, :, 0:NF8, :].transpose(3, 0, 2, 1))
    wg1b = np.ascontiguousarray(wg1_4d[:, :, NF8:, :].transpose(3, 0, 2, 1))

    return {
        "wqe8": c8(wq_pim),
        "mamb": cb(np.concatenate([MA, MB], axis=1)),
        "gvu": cb(np.concatenate([Gv * GSCL, Gu * GSCL], axis=1)),
        "gcd": cb(gcd),
        "waoT": cb(What.T),
        "woutT": cb(w_out.T),
        "w2eT": cb(W2eff.T * WGS),
        "wg18": c8(wg18),
        "wg1b": cb(wg1b),
        "negI": cb(-np.eye(H)),
        "cvec": c(cvec),
        "wvard": cb(wvar[:, None]),
        "onesd": cb(np.ones((1, H))),
        # au-token constants for host r/tr
        "_alpha": alpha, "_h_a": h_a, "_k_a": k_a,
    }


_BUILT = {}


def _get_nc():
    if "nc" not in _BUILT:
        _BUILT["nc"] = build_bass()
    return _BUILT["nc"]


def _run(inputs, trace=False):
    vf = np.asarray(inputs["visual_feat"], np.float32)
    af = np.asarray(inputs["au_feat"], np.float64)
    consts = host_constants(inputs)
    alpha = consts.pop("_alpha")
    h_a = consts.pop("_h_a")
    k_a = consts.pop("_k_a")
    bf = mybir.dt.np(BF16)
    f8 = mybir.dt.np(F8)

    # host r/tr: [17, B] -> tiled to [68, 3, B] (r | tr | au)
    t_au = af.T                                           # [17, B]
    quad = alpha * (t_au + h_a[:, None]) ** 2 + k_a[:, None]
    r17 = 1.0 / np.sqrt(quad)
    rta_full = np.empty((A4, 3, B), np.float32)
    rta_full[:, 0, :] = np.tile(r17, (NH, 1))
    rta_full[:, 1, :] = np.tile(r17 * t_au, (NH, 1))
    rta_full[:, 2, :] = np.tile(t_au, (NH, 1))
    rta_full = rta_full.astype(bf)

    # tile-major device layouts:
    # x [B, D] -> [cores, H, NT, KD, NB]: feature f = i*128+p, col = t*NB+n
    x5 = np.ascontiguousarray(
        vf.T.reshape(KD, H, NCORES, NT, NB).transpose(2, 1, 3, 0, 4))
    x5b = x5.astype(bf)
    x58 = x5.astype(f8)
    rta5 = np.ascontiguousarray(
        rta_full.reshape(A4, 3, NCORES, NT, NB).transpose(2, 0, 3, 1, 4))

    in_maps = []
    for ci in range(NCORES):
        m = dict(consts)
        m["xTb"] = x5b[ci]
        m["xT8"] = x58[ci]
        m["rta"] = rta5[ci]
        in_maps.append(m)

    nc = _get_nc()
    res = run_bass_kernel_spmd(nc, in_maps, list(range(NCORES)), trace=trace)
    out = np.empty((B, D), np.float32)
    for ci in range(NCORES):
        o = res.results[ci]["outT"].astype(np.float32)    # [H, NT, KD, NB]
        out[ci * BC:(ci + 1) * BC] = \
            o.transpose(1, 3, 2, 0).reshape(BC, D)
    return out, res


def kernel(**inputs):
    out, _ = _run(inputs, trace=False)
    return out
